# revision 2
# baseline (speedup 1.0000x reference)
"""Trainium2 Bass kernel for CommutatorConv2d.

Math: with lambda_c=0, lambda_a=1 the reference is a conv2d with effective
kernel  w_eff[o,i,r,s] = krow[o,i,s] + kcol[o,i,r]  (krow = sum_r w, kcol =
sum_s w), plus bias.  That kernel lives in a 6-dim matrix subspace, so the
9-tap conv factors into two 1D convs over box-summed inputs:

  y[o,h,w] = sum_{i,s} krow[o,i,s] * xv[i, h, w+s-1]
           + sum_{i,r} kcol[o,i,r] * xh[i, h+r-1, w]  + bias[o]

where xv = vertical 3-tap sum of zero-padded x, xh = horizontal 3-tap sum.
Per output tile that is 6 accumulating matmuls (contraction 128 each)
instead of 9 — 2/3 of the PE work of direct conv.

Sharding: data-parallel over batch; 4 images per core on 8 cores.

Startup pipeline: the scalar (Activation) engine finishes its framework
preamble ~1.3us before sync, so the image-0 head chunk + weights are issued
from scalar while sync issues the rest; dummy matmuls bridge the tensor
engine from idle to the first real tile so the HAM utilization limit never
decays mid-kernel.
"""

import os
import numpy as np
import ml_dtypes

import concourse.bass as bass
import concourse.bacc as bacc
import concourse.mybir as mybir
import concourse.tile as tile
from concourse.bass_utils import run_bass_kernel_spmd

B, CI, CO, H, W = 32, 128, 256, 56, 56
NCORES = 8
BPC = B // NCORES          # images per core
HP, WP = H + 2, W + 2      # padded spatial dims
NPIX = H * W               # 3136
ROWT = 8                   # output rows per matmul tile
NT = H // ROWT             # 7 pixel tiles per image
NTILE = ROWT * W           # 448 columns per matmul

ROW_CHUNKS0 = [10, 26, HP]  # image-0 row chunks; chunk to row r unlocks tiles t with 8t+10 <= r
N_WARM = 16                 # PE warmup matmuls (bridge idle->real work, keeps HAM limit up)
WARMC = 128                 # columns per warmup matmul

F32 = mybir.dt.float32
BF16 = mybir.dt.bfloat16


def build_nc():
    nc = bacc.Bacc(None, enable_partition_id=False)
    xin = nc.declare_dram_parameter("xp", [BPC, CI, HP, WP], BF16, isOutput=False)
    wk = nc.declare_dram_parameter("klhs", [CI, 2, 6, 128], BF16, isOutput=False)
    bb = nc.declare_dram_parameter("bias2", [CI, 2], F32, isOutput=False)
    y = nc.declare_dram_parameter("y", [BPC, CO, H, W], F32, isOutput=True)

    xflat = xin.rearrange("b c h w -> b c (h w)")
    yflat = y.rearrange("b o h w -> b o (h w)")
    wkflat = wk.rearrange("i h t o -> i (h t o)")
    NPAD = HP * WP           # 3364
    NV = H * WP              # 3248 (rows 0..55 of padded, all 58 cols)

    with tile.TileContext(nc) as tc:
        with (
            tc.tile_pool(name="const", bufs=1) as cpool,
            tc.tile_pool(name="xp", bufs=2) as xpool,
            tc.tile_pool(name="xv", bufs=2) as vpool,
            tc.tile_pool(name="xh", bufs=2) as hpool,
            tc.tile_pool(name="yo", bufs=4) as ypool,
            tc.tile_pool(name="ps", bufs=7, space="PSUM") as pspool,
        ):
            klhs_sb = cpool.tile([CI, 2 * 6 * 128], BF16)
            bias_sb = cpool.tile([CI, 2], F32)
            kl4 = klhs_sb.rearrange("i (h t o) -> i h t o", t=6, o=128)

            # PE warmup: dummy matmuls issued while the first input DMAs are
            # in flight keep the tensor engine active so the HAM utilization
            # limit ramp overlaps the DMA wait instead of the real matmuls.
            warm = cpool.tile([128, WARMC], BF16)
            nc.gpsimd.memset(warm[:], 0.0)
            warm_ps = pspool.tile([128, WARMC], F32, bufs=1, tag="warm")
            for _ in range(N_WARM):
                nc.tensor.matmul(
                    warm_ps[:], warm[:, 0:128], warm[:], start=True, stop=True
                )

            for b in range(BPC):
                row_chunks = ROW_CHUNKS0 if b == 0 else [HP]

                xp_sb = xpool.tile([CI, NPAD], BF16)
                xp3d = xflat[b].rearrange("i (h w) -> i h w", w=WP)
                xps3 = xp_sb.rearrange("i (h w) -> i h w", w=WP)
                r0 = 0
                for ci, r1 in enumerate(row_chunks):
                    if b == 0 and ci == 0:
                        # head chunk + weights from the scalar engine: it is
                        # out of the framework preamble earliest
                        nc.scalar.dma_start(out=xps3[:, r0:r1, :], in_=xp3d[:, r0:r1, :])
                        nc.scalar.dma_start(
                            out=klhs_sb[:, 0:768], in_=wkflat[:, 0:768]
                        )
                        nc.scalar.dma_start(
                            out=klhs_sb[:, 768:1536], in_=wkflat[:, 768:1536]
                        )
                        nc.scalar.dma_start(out=bias_sb[:], in_=bb[:])
                    else:
                        nc.sync.dma_start(out=xps3[:, r0:r1, :], in_=xp3d[:, r0:r1, :])
                    r0 = r1

                # box-sums, emitted per DMA chunk so they overlap the loads:
                # xv[j] = xp[j] + xp[j+58] + xp[j+116]   (rows 0..55)
                # xh[j] = xp[j] + xp[j+1] + xp[j+2]      (rows 0..57, garbage
                #                                         at cols 56/57 unused)
                xvt = vpool.tile([CI, NV], BF16)
                xv = vpool.tile([CI, NV], BF16)
                xht = hpool.tile([CI, NPAD], BF16)
                xh = hpool.tile([CI, NPAD], BF16)
                v0 = h0r = 0
                for ci, r1 in enumerate(row_chunks):
                    last = ci == len(row_chunks) - 1
                    v1 = H if last else r1 - 2        # xv rows ready
                    h1 = r1                           # xh rows ready
                    a, z = v0 * WP, v1 * WP
                    nc.vector.tensor_add(
                        xvt[:, a:z], xp_sb[:, a:z], xp_sb[:, a + WP : z + WP]
                    )
                    nc.vector.tensor_add(
                        xv[:, a:z], xvt[:, a:z], xp_sb[:, a + 2 * WP : z + 2 * WP]
                    )
                    a, z = h0r * WP, h1 * WP - 2
                    nc.vector.tensor_add(
                        xht[:, a:z], xp_sb[:, a:z], xp_sb[:, a + 1 : z + 1]
                    )
                    nc.vector.tensor_add(
                        xh[:, a:z], xht[:, a:z], xp_sb[:, a + 2 : z + 2]
                    )
                    v0, h0r = v1, h1

                xv3 = xv.rearrange("i (h w) -> i h w", w=WP)   # [128, 56, 58]
                xh3 = xh.rearrange("i (h w) -> i h w", w=WP)   # [128, 58, 58]

                youts = {}

                def emit(half, t, b=b, xv3=xv3, xh3=xh3, youts=youts):
                    if half not in youts:
                        youts[half] = ypool.tile(
                            [128, NPIX], F32, name=f"yout_{b}_{half}", tag="yout"
                        )
                    yout = youts[half]
                    h0 = t * ROWT
                    ps = pspool.tile([128, NTILE], F32, name=f"ps_{b}_{half}_{t}", tag="ps")
                    for s in range(3):
                        nc.tensor.matmul(
                            ps[:],
                            kl4[:, half, s, :],
                            xv3[:, h0 : h0 + ROWT, s : s + W],
                            start=(s == 0),
                            stop=False,
                        )
                    for r in range(3):
                        nc.tensor.matmul(
                            ps[:],
                            kl4[:, half, 3 + r, :],
                            xh3[:, h0 + r : h0 + r + ROWT, 0:W],
                            start=False,
                            stop=(r == 2),
                        )
                    last_block = b == BPC - 1 and half == 1
                    if last_block and t == NT - 1:
                        # final tile: split activation + store across the two
                        # DMA-issuing engines so the kernel tail is short
                        hw = NTILE // 2
                        c0 = t * NTILE
                        nc.scalar.activation(
                            yout[:, c0 : c0 + hw],
                            ps[:, 0:hw],
                            mybir.ActivationFunctionType.Identity,
                            bias=bias_sb[:, half : half + 1],
                        )
                        nc.sync.dma_start(
                            out=yflat[b, half * 128 : half * 128 + 128, c0 : c0 + hw],
                            in_=yout[:, c0 : c0 + hw],
                        )
                        nc.scalar.activation(
                            yout[:, c0 + hw : c0 + NTILE],
                            ps[:, hw:NTILE],
                            mybir.ActivationFunctionType.Identity,
                            bias=bias_sb[:, half : half + 1],
                        )
                        nc.scalar.dma_start(
                            out=yflat[
                                b, half * 128 : half * 128 + 128, c0 + hw : c0 + NTILE
                            ],
                            in_=yout[:, c0 + hw : c0 + NTILE],
                        )
                        return
                    nc.scalar.activation(
                        yout[:, t * NTILE : (t + 1) * NTILE],
                        ps[:],
                        mybir.ActivationFunctionType.Identity,
                        bias=bias_sb[:, half : half + 1],
                    )
                    if t == 3:
                        nc.sync.dma_start(
                            out=yflat[b, half * 128 : half * 128 + 128, 0 : 4 * NTILE],
                            in_=yout[:, 0 : 4 * NTILE],
                        )
                    elif t >= 4 and last_block:
                        # final block: per-tile stores so the kernel tail
                        # only waits on small DMAs; alternate issue engines
                        eng = nc.sync if t == 4 else nc.scalar
                        eng.dma_start(
                            out=yflat[
                                b,
                                half * 128 : half * 128 + 128,
                                t * NTILE : (t + 1) * NTILE,
                            ],
                            in_=yout[:, t * NTILE : (t + 1) * NTILE],
                        )
                    if t == NT - 1 and not last_block:
                        nc.sync.dma_start(
                            out=yflat[b, half * 128 : half * 128 + 128, 4 * NTILE : NPIX],
                            in_=yout[:, 4 * NTILE : NPIX],
                        )

                if b == 0:
                    # image 0: interleave halves so each arriving row chunk
                    # immediately unlocks two tiles of PE work
                    order = [(h, t) for t in range(NT) for h in range(2)]
                else:
                    order = [(h, t) for h in range(2) for t in range(NT)]
                for half, t in order:
                    emit(half, t)

            # read the warm PSUM bank at the very end so the warmup matmuls
            # are never dead-code-eliminated but gate nothing
            warm_out = cpool.tile([128, 32], F32)
            nc.scalar.activation(
                warm_out[:], warm_ps[:, 0:32], mybir.ActivationFunctionType.Copy
            )
    nc.finalize()
    return nc


_NC_CACHE = {}


def _get_nc():
    if "nc" not in _NC_CACHE:
        _NC_CACHE["nc"] = build_nc()
    return _NC_CACHE["nc"]


def make_in_maps(x, weight, bias):
    x = np.asarray(x, dtype=np.float32)
    weight = np.asarray(weight, dtype=np.float32)
    bias = np.asarray(bias, dtype=np.float32)

    krow = weight.sum(axis=3)  # [O, I, 3]
    kcol = weight.sum(axis=2)  # [O, I, 3]
    klhs = np.empty((CI, 2, 6, 128), np.float32)
    for half in range(2):
        o0 = half * 128
        for s in range(3):
            klhs[:, half, s, :] = krow[o0 : o0 + 128, :, s].T
            klhs[:, half, 3 + s, :] = kcol[o0 : o0 + 128, :, s].T
    klhs = klhs.astype(ml_dtypes.bfloat16)

    xp = np.zeros((B, CI, HP, WP), np.float32)
    xp[:, :, 1 : H + 1, 1 : W + 1] = x
    xp = xp.astype(ml_dtypes.bfloat16)

    bias2 = np.ascontiguousarray(bias.reshape(2, 128).T)  # [128, 2] f32

    return [
        {"xp": xp[c * BPC : (c + 1) * BPC], "klhs": klhs, "bias2": bias2}
        for c in range(NCORES)
    ]


def run(in_maps, **kwargs):
    nc = _get_nc()
    return run_bass_kernel_spmd(nc, in_maps, list(range(NCORES)), **kwargs)


def kernel(x, weight, bias):
    res = run(make_in_maps(x, weight, bias))
    return np.concatenate([res.results[c]["y"] for c in range(NCORES)], axis=0)


# revision 6
# speedup vs baseline: 1.0673x; 1.0673x over previous
"""Trainium2 Bass kernel for CommutatorConv2d.

Math: with lambda_c=0, lambda_a=1 the reference is a conv2d with effective
kernel  w_eff[o,i,r,s] = krow[o,i,s] + kcol[o,i,r]  (krow = sum_r w, kcol =
sum_s w), plus bias.  That kernel lives in a 6-dim matrix subspace, so the
9-tap conv factors into two 1D convs over box-summed inputs:

  y[o,h,w] = sum_{i,s} krow[o,i,s] * xv[i, h, w+s-1]
           + sum_{i,r} kcol[o,i,r] * xh[i, h+r-1, w]  + bias[o]

where xv = vertical 3-tap sum of zero-padded x, xh = horizontal 3-tap sum.
Per output tile that is 6 accumulating matmuls (contraction 128 each)
instead of 9 — 2/3 of the PE work of direct conv.

Sharding: data-parallel over batch; 4 images per core on 8 cores.

Startup pipeline: the scalar (Activation) engine finishes its framework
preamble ~1.3us before sync, so the image-0 head chunk + weights are issued
from scalar while sync issues the rest; dummy matmuls bridge the tensor
engine from idle to the first real tile so the HAM utilization limit never
decays mid-kernel.
"""

import os
import numpy as np
import ml_dtypes

import concourse.bass as bass
import concourse.bacc as bacc
import concourse.mybir as mybir
import concourse.tile as tile
from concourse.bass_utils import run_bass_kernel_spmd

B, CI, CO, H, W = 32, 128, 256, 56, 56
NCORES = 8
BPC = B // NCORES          # images per core
HP, WP = H + 2, W + 2      # padded spatial dims
NPIX = H * W               # 3136
ROWT = 8                   # output rows per matmul tile
NT = H // ROWT             # 7 pixel tiles per image
NTILE = ROWT * W           # 448 columns per matmul

ROW_CHUNKS0 = [10, 26, HP]  # image-0 row chunks; chunk to row r unlocks tiles t with 8t+10 <= r
N_WARM = 26                 # PE warmup matmuls (bridge idle->real work, keeps HAM limit up)
WARMC = 128                 # columns per warmup matmul

F32 = mybir.dt.float32
BF16 = mybir.dt.bfloat16


def build_nc():
    nc = bacc.Bacc(None, enable_partition_id=False)
    xin = nc.declare_dram_parameter("xp", [BPC, CI, HP, WP], BF16, isOutput=False)
    wk = nc.declare_dram_parameter("klhs", [CI, 2, 6, 128], BF16, isOutput=False)
    bb = nc.declare_dram_parameter("bias2", [CI, 2], F32, isOutput=False)
    y = nc.declare_dram_parameter("y", [BPC, CO, H, W], F32, isOutput=True)

    xflat = xin.rearrange("b c h w -> b c (h w)")
    yflat = y.rearrange("b o h w -> b o (h w)")
    wkflat = wk.rearrange("i h t o -> i (h t o)")
    NPAD = HP * WP           # 3364
    NV = H * WP              # 3248 (rows 0..55 of padded, all 58 cols)

    with tile.TileContext(nc) as tc:
        with (
            tc.tile_pool(name="const", bufs=1) as cpool,
            tc.tile_pool(name="xp", bufs=2) as xpool,
            tc.tile_pool(name="xv", bufs=2) as vpool,
            tc.tile_pool(name="xh", bufs=2) as hpool,
            tc.tile_pool(name="yo", bufs=4) as ypool,
            tc.tile_pool(name="ps", bufs=7, space="PSUM") as pspool,
        ):
            klhs_sb = cpool.tile([CI, 2 * 6 * 128], BF16)
            bias_sb = cpool.tile([CI, 2], F32)
            kl4 = klhs_sb.rearrange("i (h t o) -> i h t o", t=6, o=128)

            # PE warmup: dummy matmuls issued while the first input DMAs are
            # in flight keep the tensor engine active so the HAM utilization
            # limit ramp overlaps the DMA wait instead of the real matmuls.
            warm = cpool.tile([128, WARMC], BF16)
            nc.gpsimd.memset(warm[:], 0.0)
            warm_ps = pspool.tile([128, WARMC], F32, bufs=1, tag="warm")
            for _ in range(N_WARM):
                nc.tensor.matmul(
                    warm_ps[:], warm[:, 0:128], warm[:], start=True, stop=True
                )

            for b in range(BPC):
                row_chunks = ROW_CHUNKS0 if b == 0 else [HP]

                xp_sb = xpool.tile([CI, NPAD], BF16)
                xp3d = xflat[b].rearrange("i (h w) -> i h w", w=WP)
                xps3 = xp_sb.rearrange("i (h w) -> i h w", w=WP)
                r0 = 0
                for ci, r1 in enumerate(row_chunks):
                    # single queue, priority order: the DMA engines drain one
                    # queue's descriptors in order, so critical transfers
                    # (head chunk, first-half weights) complete first instead
                    # of round-robining with the bulk loads
                    nc.sync.dma_start(out=xps3[:, r0:r1, :], in_=xp3d[:, r0:r1, :])
                    if b == 0 and ci == 0:
                        nc.sync.dma_start(
                            out=klhs_sb[:, 0:768], in_=wkflat[:, 0:768]
                        )
                        nc.sync.dma_start(
                            out=klhs_sb[:, 768:1536], in_=wkflat[:, 768:1536]
                        )
                        nc.sync.dma_start(out=bias_sb[:], in_=bb[:])
                    r0 = r1

                # box-sums, emitted per DMA chunk so they overlap the loads:
                # xv[j] = xp[j] + xp[j+58] + xp[j+116]   (rows 0..55)
                # xh[j] = xp[j] + xp[j+1] + xp[j+2]      (rows 0..57, garbage
                #                                         at cols 56/57 unused)
                xvt = vpool.tile([CI, NV], BF16)
                xv = vpool.tile([CI, NV], BF16)
                xht = hpool.tile([CI, NPAD], BF16)
                xh = hpool.tile([CI, NPAD], BF16)
                v0 = h0r = 0
                for ci, r1 in enumerate(row_chunks):
                    last = ci == len(row_chunks) - 1
                    v1 = H if last else r1 - 2        # xv rows ready
                    h1 = r1                           # xh rows ready
                    a, z = v0 * WP, v1 * WP
                    nc.vector.tensor_add(
                        xvt[:, a:z], xp_sb[:, a:z], xp_sb[:, a + WP : z + WP]
                    )
                    nc.vector.tensor_add(
                        xv[:, a:z], xvt[:, a:z], xp_sb[:, a + 2 * WP : z + 2 * WP]
                    )
                    a, z = h0r * WP, h1 * WP - 2
                    nc.vector.tensor_add(
                        xht[:, a:z], xp_sb[:, a:z], xp_sb[:, a + 1 : z + 1]
                    )
                    nc.vector.tensor_add(
                        xh[:, a:z], xht[:, a:z], xp_sb[:, a + 2 : z + 2]
                    )
                    v0, h0r = v1, h1

                xv3 = xv.rearrange("i (h w) -> i h w", w=WP)   # [128, 56, 58]
                xh3 = xh.rearrange("i (h w) -> i h w", w=WP)   # [128, 58, 58]

                youts = {}

                def emit(half, t, b=b, xv3=xv3, xh3=xh3, youts=youts):
                    if half not in youts:
                        youts[half] = ypool.tile(
                            [128, NPIX], F32, name=f"yout_{b}_{half}", tag="yout"
                        )
                    yout = youts[half]
                    h0 = t * ROWT
                    ps = pspool.tile([128, NTILE], F32, name=f"ps_{b}_{half}_{t}", tag="ps")
                    for s in range(3):
                        nc.tensor.matmul(
                            ps[:],
                            kl4[:, half, s, :],
                            xv3[:, h0 : h0 + ROWT, s : s + W],
                            start=(s == 0),
                            stop=False,
                        )
                    for r in range(3):
                        nc.tensor.matmul(
                            ps[:],
                            kl4[:, half, 3 + r, :],
                            xh3[:, h0 + r : h0 + r + ROWT, 0:W],
                            start=False,
                            stop=(r == 2),
                        )
                    last_block = b == BPC - 1 and half == 1
                    nc.scalar.activation(
                        yout[:, t * NTILE : (t + 1) * NTILE],
                        ps[:],
                        mybir.ActivationFunctionType.Identity,
                        bias=bias_sb[:, half : half + 1],
                    )
                    if t == 3:
                        nc.sync.dma_start(
                            out=yflat[b, half * 128 : half * 128 + 128, 0 : 4 * NTILE],
                            in_=yout[:, 0 : 4 * NTILE],
                        )
                    elif t >= 4 and last_block:
                        # final block: per-tile stores so the kernel tail
                        # only waits on small DMAs
                        nc.sync.dma_start(
                            out=yflat[
                                b,
                                half * 128 : half * 128 + 128,
                                t * NTILE : (t + 1) * NTILE,
                            ],
                            in_=yout[:, t * NTILE : (t + 1) * NTILE],
                        )
                    if t == NT - 1 and not last_block:
                        nc.sync.dma_start(
                            out=yflat[b, half * 128 : half * 128 + 128, 4 * NTILE : NPIX],
                            in_=yout[:, 4 * NTILE : NPIX],
                        )

                if b == 0:
                    # image 0: interleave halves so each arriving row chunk
                    # immediately unlocks two tiles of PE work
                    order = [(h, t) for t in range(NT) for h in range(2)]
                else:
                    order = [(h, t) for h in range(2) for t in range(NT)]
                for half, t in order:
                    emit(half, t)

            # read the warm PSUM bank at the very end so the warmup matmuls
            # are never dead-code-eliminated but gate nothing
            warm_out = cpool.tile([128, 32], F32)
            nc.scalar.activation(
                warm_out[:], warm_ps[:, 0:32], mybir.ActivationFunctionType.Copy
            )
    nc.finalize()
    return nc


_NC_CACHE = {}


def _get_nc():
    if "nc" not in _NC_CACHE:
        _NC_CACHE["nc"] = build_nc()
    return _NC_CACHE["nc"]


def make_in_maps(x, weight, bias):
    x = np.asarray(x, dtype=np.float32)
    weight = np.asarray(weight, dtype=np.float32)
    bias = np.asarray(bias, dtype=np.float32)

    krow = weight.sum(axis=3)  # [O, I, 3]
    kcol = weight.sum(axis=2)  # [O, I, 3]
    klhs = np.empty((CI, 2, 6, 128), np.float32)
    for half in range(2):
        o0 = half * 128
        for s in range(3):
            klhs[:, half, s, :] = krow[o0 : o0 + 128, :, s].T
            klhs[:, half, 3 + s, :] = kcol[o0 : o0 + 128, :, s].T
    klhs = klhs.astype(ml_dtypes.bfloat16)

    xp = np.zeros((B, CI, HP, WP), np.float32)
    xp[:, :, 1 : H + 1, 1 : W + 1] = x
    xp = xp.astype(ml_dtypes.bfloat16)

    bias2 = np.ascontiguousarray(bias.reshape(2, 128).T)  # [128, 2] f32

    return [
        {"xp": xp[c * BPC : (c + 1) * BPC], "klhs": klhs, "bias2": bias2}
        for c in range(NCORES)
    ]


def run(in_maps, **kwargs):
    nc = _get_nc()
    return run_bass_kernel_spmd(nc, in_maps, list(range(NCORES)), **kwargs)


def kernel(x, weight, bias):
    res = run(make_in_maps(x, weight, bias))
    return np.concatenate([res.results[c]["y"] for c in range(NCORES)], axis=0)


# revision 10
# speedup vs baseline: 1.0774x; 1.0095x over previous
"""Trainium2 Bass kernel for CommutatorConv2d.

Math: with lambda_c=0, lambda_a=1 the reference is a conv2d with effective
kernel  w_eff[o,i,r,s] = krow[o,i,s] + kcol[o,i,r]  (krow = sum_r w, kcol =
sum_s w), plus bias.  That kernel lives in a 6-dim matrix subspace, so the
9-tap conv factors into two 1D convs over box-summed inputs:

  y[o,h,w] = sum_{i,s} krow[o,i,s] * xv[i, h, w+s-1]
           + sum_{i,r} kcol[o,i,r] * xh[i, h+r-1, w]  + bias[o]

where xv = vertical 3-tap sum of zero-padded x, xh = horizontal 3-tap sum.
Per output tile that is 6 accumulating matmuls (contraction 128 each)
instead of 9 — 2/3 of the PE work of direct conv.

Sharding: data-parallel over batch; 4 images per core on 8 cores.

Startup pipeline: the scalar (Activation) engine finishes its framework
preamble ~1.3us before sync, so the image-0 head chunk + weights are issued
from scalar while sync issues the rest; dummy matmuls bridge the tensor
engine from idle to the first real tile so the HAM utilization limit never
decays mid-kernel.
"""

import os
import numpy as np
import ml_dtypes

import concourse.bass as bass
import concourse.bacc as bacc
import concourse.mybir as mybir
import concourse.tile as tile
from concourse.bass_utils import run_bass_kernel_spmd

B, CI, CO, H, W = 32, 128, 256, 56, 56
NCORES = 8
BPC = B // NCORES          # images per core
HP, WP = H + 2, W + 2      # padded spatial dims
NPIX = H * W               # 3136
ROWT = 8                   # output rows per matmul tile
NT = H // ROWT             # 7 pixel tiles per image
NTILE = ROWT * W           # 448 columns per matmul

ROW_CHUNKS0 = [10, 26, HP]  # image-0 row chunks; chunk to row r unlocks tiles t with 8t+10 <= r
# box-sum sub-splits per chunk: compute xv/xh for the first unlocked tile's
# rows first so the PE never waits on a full-chunk vector op
SUM_SPLITS = {26: [18, 26], HP: [42, HP]}
N_WARM = 33                 # PE warmup matmuls (bridge idle->real work, keeps HAM limit up)
WARMC = 128                 # columns per warmup matmul

F32 = mybir.dt.float32
BF16 = mybir.dt.bfloat16


def build_nc():
    nc = bacc.Bacc(None, enable_partition_id=False)
    xin = nc.declare_dram_parameter("xp", [BPC, CI, HP, WP], BF16, isOutput=False)
    wk = nc.declare_dram_parameter("klhs", [CI, 2, 6, 128], BF16, isOutput=False)
    bb = nc.declare_dram_parameter("bias2", [CI, 2], F32, isOutput=False)
    y = nc.declare_dram_parameter("y", [BPC, CO, H, W], BF16, isOutput=True)

    xflat = xin.rearrange("b c h w -> b c (h w)")
    yflat = y.rearrange("b o h w -> b o (h w)")
    wkflat = wk.rearrange("i h t o -> i (h t o)")
    NPAD = HP * WP           # 3364
    NV = H * WP              # 3248 (rows 0..55 of padded, all 58 cols)

    with tile.TileContext(nc) as tc:
        with (
            tc.tile_pool(name="const", bufs=1) as cpool,
            tc.tile_pool(name="xp", bufs=2) as xpool,
            tc.tile_pool(name="xv", bufs=2) as vpool,
            tc.tile_pool(name="xh", bufs=2) as hpool,
            tc.tile_pool(name="yo", bufs=4) as ypool,
            tc.tile_pool(name="ps", bufs=7, space="PSUM") as pspool,
        ):
            klhs_sb = cpool.tile([CI, 2 * 6 * 128], BF16)
            bias_sb = cpool.tile([CI, 2], F32)
            kl4 = klhs_sb.rearrange("i (h t o) -> i h t o", t=6, o=128)

            # PE warmup: dummy matmuls issued while the first input DMAs are
            # in flight keep the tensor engine active so the HAM utilization
            # limit ramp overlaps the DMA wait instead of the real matmuls.
            warm = cpool.tile([128, WARMC], BF16)
            nc.gpsimd.memset(warm[:], 0.0)
            warm_ps = pspool.tile([128, WARMC], F32, bufs=1, tag="warm")
            for _ in range(N_WARM):
                nc.tensor.matmul(
                    warm_ps[:], warm[:, 0:128], warm[:], start=True, stop=True
                )

            for b in range(BPC):
                row_chunks = ROW_CHUNKS0 if b == 0 else [HP]

                xp_sb = xpool.tile([CI, NPAD], BF16)
                xp3d = xflat[b].rearrange("i (h w) -> i h w", w=WP)
                xps3 = xp_sb.rearrange("i (h w) -> i h w", w=WP)
                r0 = 0
                for ci, r1 in enumerate(row_chunks):
                    # single queue, priority order: the DMA engines drain one
                    # queue's descriptors in order, so critical transfers
                    # (head chunk, first-half weights) complete first instead
                    # of round-robining with the bulk loads
                    nc.sync.dma_start(out=xps3[:, r0:r1, :], in_=xp3d[:, r0:r1, :])
                    if b == 0 and ci == 0:
                        nc.sync.dma_start(
                            out=klhs_sb[:, 0:768], in_=wkflat[:, 0:768]
                        )
                        nc.sync.dma_start(
                            out=klhs_sb[:, 768:1536], in_=wkflat[:, 768:1536]
                        )
                        nc.sync.dma_start(out=bias_sb[:], in_=bb[:])
                    r0 = r1

                # box-sums, emitted per DMA chunk so they overlap the loads:
                # xv[j] = xp[j] + xp[j+58] + xp[j+116]   (rows 0..55)
                # xh[j] = xp[j] + xp[j+1] + xp[j+2]      (rows 0..57, garbage
                #                                         at cols 56/57 unused)
                xvt = vpool.tile([CI, NV], BF16)
                xv = vpool.tile([CI, NV], BF16)
                xht = hpool.tile([CI, NPAD], BF16)
                xh = hpool.tile([CI, NPAD], BF16)
                bounds = []
                for r1 in row_chunks:
                    bounds.extend(SUM_SPLITS.get(r1, [r1]) if b == 0 else [r1])
                v0 = h0r = 0
                for s1 in bounds:
                    v1 = H if s1 == HP else s1 - 2    # xv rows ready
                    h1 = s1                           # xh rows ready
                    a, z = v0 * WP, v1 * WP
                    nc.vector.tensor_add(
                        xvt[:, a:z], xp_sb[:, a:z], xp_sb[:, a + WP : z + WP]
                    )
                    nc.vector.tensor_add(
                        xv[:, a:z], xvt[:, a:z], xp_sb[:, a + 2 * WP : z + 2 * WP]
                    )
                    a, z = h0r * WP, h1 * WP - 2
                    nc.vector.tensor_add(
                        xht[:, a:z], xp_sb[:, a:z], xp_sb[:, a + 1 : z + 1]
                    )
                    nc.vector.tensor_add(
                        xh[:, a:z], xht[:, a:z], xp_sb[:, a + 2 : z + 2]
                    )
                    v0, h0r = v1, h1

                xv3 = xv.rearrange("i (h w) -> i h w", w=WP)   # [128, 56, 58]
                xh3 = xh.rearrange("i (h w) -> i h w", w=WP)   # [128, 58, 58]

                youts = {}

                def emit(half, t, b=b, xv3=xv3, xh3=xh3, youts=youts):
                    if half not in youts:
                        youts[half] = ypool.tile(
                            [128, NPIX], BF16, name=f"yout_{b}_{half}", tag="yout"
                        )
                    yout = youts[half]
                    h0 = t * ROWT
                    ps = pspool.tile([128, NTILE], F32, name=f"ps_{b}_{half}_{t}", tag="ps")
                    for s in range(3):
                        nc.tensor.matmul(
                            ps[:],
                            kl4[:, half, s, :],
                            xv3[:, h0 : h0 + ROWT, s : s + W],
                            start=(s == 0),
                            stop=False,
                        )
                    for r in range(3):
                        nc.tensor.matmul(
                            ps[:],
                            kl4[:, half, 3 + r, :],
                            xh3[:, h0 + r : h0 + r + ROWT, 0:W],
                            start=False,
                            stop=(r == 2),
                        )
                    last_block = b == BPC - 1 and half == 1
                    if last_block and t == NT - 1:
                        # final tile: split activation + store into halves so
                        # the second store's issue overlaps the first's and
                        # the kernel tail only waits on a 224-column DMA
                        c0 = t * NTILE
                        for p0, p1 in ((0, NTILE // 2), (NTILE // 2, NTILE)):
                            nc.scalar.activation(
                                yout[:, c0 + p0 : c0 + p1],
                                ps[:, p0:p1],
                                mybir.ActivationFunctionType.Identity,
                                bias=bias_sb[:, half : half + 1],
                            )
                            nc.sync.dma_start(
                                out=yflat[
                                    b, half * 128 : half * 128 + 128, c0 + p0 : c0 + p1
                                ],
                                in_=yout[:, c0 + p0 : c0 + p1],
                            )
                        return
                    nc.scalar.activation(
                        yout[:, t * NTILE : (t + 1) * NTILE],
                        ps[:],
                        mybir.ActivationFunctionType.Identity,
                        bias=bias_sb[:, half : half + 1],
                    )
                    if t == 3:
                        nc.sync.dma_start(
                            out=yflat[b, half * 128 : half * 128 + 128, 0 : 4 * NTILE],
                            in_=yout[:, 0 : 4 * NTILE],
                        )
                    elif t >= 4 and last_block:
                        # final block: per-tile stores so the kernel tail
                        # only waits on small DMAs
                        nc.sync.dma_start(
                            out=yflat[
                                b,
                                half * 128 : half * 128 + 128,
                                t * NTILE : (t + 1) * NTILE,
                            ],
                            in_=yout[:, t * NTILE : (t + 1) * NTILE],
                        )
                    if t == NT - 1 and not last_block:
                        nc.sync.dma_start(
                            out=yflat[b, half * 128 : half * 128 + 128, 4 * NTILE : NPIX],
                            in_=yout[:, 4 * NTILE : NPIX],
                        )

                if b == 0:
                    # image 0: interleave halves so each arriving row chunk
                    # immediately unlocks two tiles of PE work
                    order = [(h, t) for t in range(NT) for h in range(2)]
                else:
                    order = [(h, t) for h in range(2) for t in range(NT)]
                for half, t in order:
                    emit(half, t)

            # read the warm PSUM bank at the very end so the warmup matmuls
            # are never dead-code-eliminated but gate nothing
            warm_out = cpool.tile([128, 32], F32)
            nc.scalar.activation(
                warm_out[:], warm_ps[:, 0:32], mybir.ActivationFunctionType.Copy
            )
    nc.finalize()
    return nc


_NC_CACHE = {}


def _get_nc():
    if "nc" not in _NC_CACHE:
        _NC_CACHE["nc"] = build_nc()
    return _NC_CACHE["nc"]


def make_in_maps(x, weight, bias):
    x = np.asarray(x, dtype=np.float32)
    weight = np.asarray(weight, dtype=np.float32)
    bias = np.asarray(bias, dtype=np.float32)

    krow = weight.sum(axis=3)  # [O, I, 3]
    kcol = weight.sum(axis=2)  # [O, I, 3]
    klhs = np.empty((CI, 2, 6, 128), np.float32)
    for half in range(2):
        o0 = half * 128
        for s in range(3):
            klhs[:, half, s, :] = krow[o0 : o0 + 128, :, s].T
            klhs[:, half, 3 + s, :] = kcol[o0 : o0 + 128, :, s].T
    klhs = klhs.astype(ml_dtypes.bfloat16)

    xp = np.zeros((B, CI, HP, WP), np.float32)
    xp[:, :, 1 : H + 1, 1 : W + 1] = x
    xp = xp.astype(ml_dtypes.bfloat16)

    bias2 = np.ascontiguousarray(bias.reshape(2, 128).T)  # [128, 2] f32

    return [
        {"xp": xp[c * BPC : (c + 1) * BPC], "klhs": klhs, "bias2": bias2}
        for c in range(NCORES)
    ]


def run(in_maps, **kwargs):
    nc = _get_nc()
    return run_bass_kernel_spmd(nc, in_maps, list(range(NCORES)), **kwargs)


def kernel(x, weight, bias):
    res = run(make_in_maps(x, weight, bias))
    return np.concatenate(
        [res.results[c]["y"].astype(np.float32) for c in range(NCORES)], axis=0
    )
